# revision 10
# baseline (speedup 1.0000x reference)
"""Causal MHA attention-out kernel for TRN2, head-sharded across 8 NeuronCores.

Reference computation (fp32):
    scores = (q @ k^T) / sqrt(64), causal mask, softmax
    z      = pattern @ v
    out    = sum_h z_h @ W_O[h] + b_O          (residual passed through)

Sharding: 16 heads -> 8 cores x 2 adjacent heads. Each core computes a
partial out (its 2 heads' contribution, both batches); host sums partials.

Per-core layout (per batch b):
  kT/qT  [128, 2048]   d-major (head0 -> partitions 0-63, head1 -> 64-127),
                       loaded via bf16 xbar DMA transpose (dma_start_transpose).
  scores  [128 k, 1024] per k-block — BOTH heads side by side (h0 cols
          0:512, h1 cols 512:1024) in one 2-bank PSUM tile (tag "sc", 2
          slots = two k-steps in flight); heads use disjoint PE row halves.
  softmax: ONE 1024-wide exp per k-block on ACT straight out of PSUM
           (scale=1/8 folded in) — wide insts amortize the ~370-cycle ACT
           access overhead; causal triangles of both heads zeroed by one
           strided bf16 DVE multiply; denominator comes from a ones-column
           folded into V (row 64 of zT).
  z^T via matmul(lhsT=v_aug, rhs=patternT); normalize via DVE reciprocal +
           gpsimd partition_broadcast; project with both heads stacked (K=128).
  Software-pipelined emission: PV lags QK/exp by 2 k-steps so the in-order PE
  stream never blocks behind ACT; each chunk's normalize/projection/output
  tail is drained piecewise inside the NEXT chunk's step loop. The two
  batches interleave at q-chunk granularity. Output stages into a wide
  [128, 4096] SBUF tile -> one coalesced DMA per 512-row chunk.
"""

import numpy as np

import concourse.bass as bass
import concourse.mybir as mybir
from concourse import bacc
import concourse.tile as tile
from concourse.bass_utils import run_bass_kernel_spmd

B = 2
S = 2048
D_MODEL = 1024
N_HEADS = 16
D_HEAD = 64
N_CORES = 8
HPC = 2  # heads per core
CW = HPC * D_HEAD  # 128 columns of q/k/v per core
NKB = S // 128  # 16 k-blocks
NQC = S // 512  # 4 q-chunks
INV_SCALE = 1.0 / 8.0  # 1/sqrt(64)
LAG = 2  # k-steps PV trails QK/exp in the emission pipeline

F32 = mybir.dt.float32
MMDT = mybir.dt.bfloat16  # matmul operand dtype: guaranteed 1 cyc/row on PE

_CACHE = {}


def _build_bass(reps=None):
    nc = bacc.Bacc("TRN2", target_bir_lowering=False)

    q_d = nc.dram_tensor("q", [B, S, CW], MMDT, kind="ExternalInput")
    k_d = nc.dram_tensor("k", [B, S, CW], MMDT, kind="ExternalInput")
    v_d = nc.dram_tensor("v", [B, S, CW], MMDT, kind="ExternalInput")
    wo_d = nc.dram_tensor("wo", [CW, D_MODEL], MMDT, kind="ExternalInput")
    out_d = nc.dram_tensor("out", [B, S, D_MODEL], MMDT, kind="ExternalOutput")

    with tile.TileContext(nc) as tc:
        with (
            tc.tile_pool(name="const", bufs=1) as const_pool,
            tc.tile_pool(name="big", bufs=2) as big_pool,
            tc.tile_pool(name="stage", bufs=4) as stage_pool,
            tc.tile_pool(name="pat", bufs=6) as pat_pool,
            tc.tile_pool(name="osb", bufs=3) as osb_pool,
            tc.tile_pool(name="psc", bufs=2, space="PSUM") as psc_pool,
            tc.tile_pool(name="pz", bufs=2, space="PSUM") as pz_pool,
        ):
            tri_f = const_pool.tile([128, 128], F32)
            nc.gpsimd.memset(tri_f, 1.0)
            nc.gpsimd.affine_select(
                out=tri_f,
                in_=tri_f,
                compare_op=mybir.AluOpType.is_ge,
                fill=0.0,
                base=0,
                pattern=[[1, 128]],
                channel_multiplier=-1,
            )
            tri = const_pool.tile([128, 128], MMDT)
            nc.vector.tensor_copy(tri, tri_f)
            tri2 = const_pool.tile([128, 256], MMDT)
            nc.vector.tensor_copy(tri2[:, 0:128], tri_f)
            nc.vector.tensor_copy(tri2[:, 128:256], tri_f)
            wo_sb = const_pool.tile([CW, D_MODEL], MMDT)
            nc.sync.dma_start(wo_sb, wo_d[:, :])

            import contextlib

            loop_cm = (
                tc.For_i(
                    0,
                    reps,
                    1,
                    hint_engines=(
                        mybir.EngineType.PE,
                        mybir.EngineType.DVE,
                        mybir.EngineType.Activation,
                        mybir.EngineType.Pool,
                        mybir.EngineType.SP,
                    ),
                    staggered_reset=True,
                )
                if reps
                else contextlib.nullcontext()
            )
            with loop_cm:
                _emit_body(nc, tc, locals())
    nc.compile()
    return nc


def _emit_body(nc, tc, env):
    (q_d, k_d, v_d, wo_d, out_d) = (
        env["q_d"], env["k_d"], env["v_d"], env["wo_d"], env["out_d"]
    )
    (const_pool, big_pool, stage_pool, pat_pool, osb_pool, psc_pool, pz_pool) = (
        env["const_pool"], env["big_pool"], env["stage_pool"], env["pat_pool"],
        env["osb_pool"], env["psc_pool"], env["pz_pool"]
    )
    wo_sb, tri, tri2 = env["wo_sb"], env["tri"], env["tri2"]

    kTs, qTs, vbigs = [], [], []
    for b in range(B):
        kT = big_pool.tile([128, S], MMDT, tag="kT", name=f"kT{b}")
        qT = big_pool.tile([128, S], MMDT, tag="qT", name=f"qT{b}")
        # v packed per k-block as [v_h0 | ones | v_h1 | ones] (130 cols)
        vbig = big_pool.tile([128, NKB * 130], MMDT, tag="vb", name=f"vb{b}")
        kTs.append(kT); qTs.append(qT); vbigs.append(vbig)

    # loads ordered so qc<2 of both batches unblock earliest: first halves
    # (k/q positions 0:1024) of b0 and b1, then second halves
    for b in range(B):
        nc.sync.dma_start_transpose(kTs[b][:, 0:1024], k_d[b, 0:1024, :])
        nc.sync.dma_start_transpose(qTs[b][:, 0:1024], q_d[b, 0:1024, :])
        v4 = vbigs[b].rearrange("p (t r c) -> p t r c", r=2, c=65)
        v3 = vbigs[b].rearrange("p (t c) -> p t c", c=130)
        nc.sync.dma_start(
            v3[:, :, 0:64],
            v_d[b].rearrange("(t p) c -> p t c", p=128)[:, :, 0:64],
        )
        nc.sync.dma_start(
            v3[:, :, 65:129],
            v_d[b].rearrange("(t p) c -> p t c", p=128)[:, :, 64:128],
        )
        nc.gpsimd.memset(v4[:, :, :, 64], 1.0)
    for b in range(B):
        nc.sync.dma_start_transpose(kTs[b][:, 1024:2048], k_d[b, 1024:2048, :])
        nc.sync.dma_start_transpose(qTs[b][:, 1024:2048], q_d[b, 1024:2048, :])

    def make_chunk(b, qc):
        """Emission closures for one (batch, q-chunk)."""
        kT, qT, vbig = kTs[b], qTs[b], vbigs[b]
        steps = 4 * qc + 4
        last_kb = steps - 1
        scs, pats = {}, {}
        state = {}

        def s_off(kb):
            dd = kb - 4 * qc
            return 128 * dd if dd > 0 else 0

        def qk(kb):
            so = s_off(kb)
            # both heads' scores for this k-block in ONE 2-bank PSUM tile
            # (h0 cols 0:512, h1 cols 512:1024) so exp is a single wide inst
            sc = psc_pool.tile([128, 1024], F32, tag="sc", name=f"sc{b}_{qc}_{kb}")
            scs[kb] = sc
            for h in range(HPC):
                nc.tensor.matmul(
                    sc[:, h * 512 + so : h * 512 + 512],
                    lhsT=kT[64 * h : 64 * h + 64, kb * 128 : (kb + 1) * 128],
                    rhs=qT[64 * h : 64 * h + 64, qc * 512 + so : (qc + 1) * 512],
                    start=True,
                    stop=True,
                )

        def expmask(kb):
            so = s_off(kb)
            dd = kb - 4 * qc
            pt = pat_pool.tile([128, 1024], MMDT, tag="pat", name=f"pat{b}_{qc}_{kb}")
            pats[kb] = pt
            # single span [so:1024]; the [512:512+so) gap is never written by
            # QK nor read by PV — exp of stale PSUM there is harmless
            nc.scalar.activation(
                pt[:, so:1024],
                scs[kb][:, so:1024],
                mybir.ActivationFunctionType.Exp,
                scale=INV_SCALE,
            )
            if dd >= 0:
                # causal triangles on the idle Pool engine: a mask waiting on
                # exp would head-of-line-block queued projection drains in
                # the strict-FIFO DVE queue
                for h in range(HPC):
                    ap = pt[:, h * 512 + dd * 128 : h * 512 + dd * 128 + 128]
                    nc.gpsimd.affine_select(
                        out=ap, in_=ap,
                        compare_op=mybir.AluOpType.is_ge,
                        fill=0.0, base=0, pattern=[[1, 128]],
                        channel_multiplier=-1,
                    )

        def pv(kb):
            if kb == 0:
                state["zacc"] = [
                    pz_pool.tile([65, 512], F32, tag="z", name=f"zacc{b}_{qc}_{h}")
                    for h in range(HPC)
                ]
            so = s_off(kb)
            for h in range(HPC):
                nc.tensor.matmul(
                    state["zacc"][h][:, so:512],
                    lhsT=vbig[:, kb * 130 + 65 * h : kb * 130 + 65 * h + 65],
                    rhs=pats[kb][:, h * 512 + so : h * 512 + 512],
                    start=(kb == 0),
                    stop=(kb == last_kb),
                )

        def tail_pieces():
        # normalize + project + stage + writeback, split into drainable bits
            zacc = state["zacc"]
            zsb = stage_pool.tile([128, 512], MMDT, tag="zsb", name=f"zsb{b}_{qc}")
            rbs = {}

            def t_recip():
                for h in range(HPC):
                    r_sb = stage_pool.tile([1, 512], F32, tag="r")
                    nc.vector.reciprocal(r_sb, zacc[h][64:65, :])
                    rb = stage_pool.tile([64, 512], F32, tag="rb")
                    nc.gpsimd.partition_broadcast(rb, r_sb)
                    rbs[h] = rb

            def t_norm():
                for h in range(HPC):
                    nc.vector.tensor_mul(
                        zsb[64 * h : 64 * h + 64, :], zacc[h][0:64, :], rbs[h]
                    )

            state["osb"] = None

            def mk_proj(qb):
                def t_proj():
                    if state["osb"] is None:
                        state["osb"] = osb_pool.tile(
                            [128, 4 * D_MODEL], MMDT, tag="osb", name=f"osb{b}_{qc}"
                        )
                    osb = state["osb"]
                    for mch in range(2):
                        op = pz_pool.tile(
                            [128, 512], F32, tag="op",
                            name=f"op{b}_{qc}_{qb}_{mch}",
                        )
                        nc.tensor.matmul(
                            op,
                            lhsT=zsb[:, qb * 128 : (qb + 1) * 128],
                            rhs=wo_sb[:, mch * 512 : (mch + 1) * 512],
                            start=True,
                            stop=True,
                        )
                        dst = osb[:, qb * D_MODEL + mch * 512 : qb * D_MODEL + mch * 512 + 512]
                        nc.vector.tensor_copy(dst, op)
                return t_proj

            def t_dma():
                r0 = qc * 512
                nc.sync.dma_start(
                    out_d[b, r0 : r0 + 512, :].rearrange("(t p) m -> p t m", p=128),
                    state["osb"].rearrange("p (t m) -> p t m", m=D_MODEL),
                )

            return [t_recip, t_norm, mk_proj(0), mk_proj(1), mk_proj(2), mk_proj(3), t_dma]

        return steps, qk, expmask, pv, tail_pieces

    chunks = [make_chunk(b, qc) for qc in range(NQC) for b in range(B)]

    pending = []
    for steps, qk, expmask, pv, tail_pieces in chunks:
        for s in range(steps):
            qk(s)
            expmask(s)
            if s >= LAG:
                pv(s - LAG)
            # drain up to 2 pieces of the previous chunk's tail per step
            for _ in range(2):
                if pending:
                    pending.pop(0)()
        for s in range(steps - LAG, steps):
            pv(s)
        while pending:
            pending.pop(0)()
        pending = tail_pieces()
    while pending:
        pending.pop(0)()


def make_in_maps(q, k, v, W_O):
    import ml_dtypes

    bf16 = ml_dtypes.bfloat16
    q = np.asarray(q, dtype=np.float32).astype(bf16)
    k = np.asarray(k, dtype=np.float32).astype(bf16)
    v = np.asarray(v, dtype=np.float32).astype(bf16)
    W_O = np.asarray(W_O, dtype=np.float32).astype(bf16)
    in_maps = []
    for c in range(N_CORES):
        cols = slice(c * CW, (c + 1) * CW)
        in_maps.append(
            {
                "q": np.ascontiguousarray(q[:, :, cols]),
                "k": np.ascontiguousarray(k[:, :, cols]),
                "v": np.ascontiguousarray(v[:, :, cols]),
                "wo": np.ascontiguousarray(
                    W_O[c * HPC : (c + 1) * HPC].reshape(CW, D_MODEL)
                ),
            }
        )
    return in_maps


def get_nc():
    if "nc" not in _CACHE:
        _CACHE["nc"] = _build_bass()
    return _CACHE["nc"]


def kernel(q, k, v, residual, W_O, b_O):
    nc = get_nc()
    in_maps = make_in_maps(q, k, v, W_O)
    res = run_bass_kernel_spmd(nc, in_maps, core_ids=list(range(N_CORES)))
    out = res.results[0]["out"].astype(np.float64)
    for r in res.results[1:]:
        out += r["out"].astype(np.float64)
    out = (out + np.asarray(b_O, dtype=np.float64)[None, None, :]).astype(np.float32)
    return out, np.asarray(residual)
